# revision 11
# baseline (speedup 1.0000x reference)
"""Trainium2 Bass kernel for CombinedRepeatCausalLinear (parallel forward).

Computes out[b,e,t] = sum_s x[b,e,s] * W[s,t] + bias[t] where
  W[s,t] = mask(t>=s) * (w0[s]*d0^(t-s) + w1[t]*d1^(t-s))
for S = 2048, x of shape (8, 1024, 2048) fp32.

Strategy (8 NeuronCores, data-parallel over batch; fp16 datapath):
  W is causal-masked rank-2.  Split s/t into 17 chunks of C=126.  For
  target chunk J the contribution of all s < 126J is exactly rank 2:
     out[t in J] = (diag block) + d0^tl * A0_J + w1[t] d1^tl * A1_J
  with A0_J[e] = sum_{s<126J} w0[s] d0^(126J-s) x[s,e]  (A1 analogous).
  C=126 leaves 2 spare K-rows, so the cross term folds into the SAME
  K=128 matmul as the 126x126 diagonal block: moving-operand partitions
  0/1 carry the per-chunk A rows, partitions 2..127 carry the x chunk;
  the stationary's rows 0/1 are the decay rows.

  v3 pipeline: all stationaries (wf) are host-computed (no on-device
  wgen — its PSUM->DVE chain serialized v2).  Small params land first
  on the HWDGE rings, then 9 paired x chunk DMAs balanced across the
  sync/scalar rings; per-chunk A-contribution matmuls (4-strip
  col-tiled PSUM accumulation) hide under the load.  A_1..A_15 need
  only chunks 0..14, so their strip reduce + 4-way split scatter is
  emitted BETWEEN chunk 14's and chunk 15's A matmuls; A_16 comes from
  a tiny pass B.  Mains stream per chunk with PSUM->SBUF bias copies
  split ACT/DVE and paired 516KB output DMAs on the sync/gpsimd rings.
  fp16 in SBUF; fp32 PSUM accumulate.
"""

import numpy as np

import concourse.bass as bass
import concourse.mybir as mybir
import concourse.tile as tile
from concourse import bacc
from concourse.bass_utils import run_bass_kernel_spmd

F16 = mybir.dt.float16
F32 = mybir.dt.float32

B = 8
E = 1024
S = 2048
DC = 1.0
N_CORES = 8
R = (B * E) // N_CORES      # rows (e) per core = 1024
C = 126                     # chunk size along s/t
NCH = 17                    # chunks; chunk 16 has only 32 valid rows
LAST = S - C * (NCH - 1)    # 32

_PROGRAM = None


def _build_program():
    nc = bacc.Bacc("TRN2", target_bir_lowering=False, debug=False,
                   num_devices=N_CORES)

    xg_d = nc.declare_dram_parameter("xg", [128, NCH * R], F16, isOutput=False)
    # p1: uu(512) | predq(32) | predb(2) | wf0(126) cols, fp16
    p1_d = nc.declare_dram_parameter("p1", [128, 672], F16, isOutput=False)
    wfa_d = nc.declare_dram_parameter("wfa", [128, 8 * C], F16, isOutput=False)
    wfb_d = nc.declare_dram_parameter("wfb", [128, 8 * C], F16, isOutput=False)
    biasT_d = nc.declare_dram_parameter("biasT", [C, NCH], F32,
                                        isOutput=False)
    outg_d = nc.declare_dram_parameter("outg", [C, NCH * R], F16,
                                       isOutput=True)

    Ident = mybir.ActivationFunctionType.Identity

    with tile.TileContext(nc) as tc:
        with (
            tc.tile_pool(name="cst", bufs=1) as cst,
            tc.tile_pool(name="xp", bufs=1) as xp,
            tc.tile_pool(name="osb", bufs=3) as osb,
            tc.tile_pool(name="ps", bufs=1, space="PSUM") as psp,
            tc.tile_pool(name="po", bufs=2, space="PSUM") as pop,
        ):
            # --- params first on the HWDGE rings (SWDGE starves behind
            # bulk HWDGE traffic), then x pairs, then the bulk wf blocks
            p1_sb = cst.tile([128, 672], F16, tag="p1")
            nc.sync.dma_start(p1_sb[:], p1_d[:])
            bias_sb = cst.tile([C, NCH], F32, tag="bias")
            nc.scalar.dma_start(bias_sb[:], biasT_d[:])
            uu_sb = p1_sb[:, 0:512]
            predq_sb = p1_sb[:, 512:544]
            predb_sb = p1_sb[:, 544:546]
            wf0_sb = p1_sb[:, 546:672]

            xg = xp.tile([128, NCH * R], F16, tag="xg")
            sync_pairs = [(0, 2), (4, 6), (8, 10), (12, 14), (16, 17)]
            scalar_pairs = [(2, 4), (6, 8), (10, 12), (14, 16)]
            for i in range(5):
                lo, hi = sync_pairs[i]
                nc.sync.dma_start(xg[:, lo * R:hi * R], xg_d[:, lo * R:hi * R])
                if i < 4:
                    lo, hi = scalar_pairs[i]
                    nc.scalar.dma_start(xg[:, lo * R:hi * R],
                                        xg_d[:, lo * R:hi * R])
            wfa_sb = cst.tile([128, 8 * C], F16, tag="wfa")
            nc.scalar.dma_start(wfa_sb[:], wfa_d[:])
            wfb_sb = cst.tile([128, 8 * C], F16, tag="wfb")
            nc.scalar.dma_start(wfb_sb[:], wfb_d[:])

            def wf(J):
                if J == 0:
                    return wf0_sb
                if J <= 8:
                    return wfa_sb[:, (J - 1) * C:J * C]
                return wfb_sb[:, (J - 9) * C:(J - 8) * C]

            def emit_main(J, out_sb, col0, eng_dma=None, dma_src=None,
                          dma_dst=None):
                po = pop.tile([C, R], F32, tag="po", name=f"po{J}")
                for h in range(2):
                    nc.tensor.matmul(po[:, 512 * h:512 * (h + 1)], wf(J),
                                     xg[:, R * J + 512 * h:
                                        R * J + 512 * (h + 1)],
                                     start=True, stop=True)
                nc.scalar.activation(out_sb[:, col0:col0 + 512],
                                     po[:, 0:512], Ident,
                                     bias=bias_sb[:, J:J + 1])
                nc.vector.tensor_scalar_add(out_sb[:, col0 + 512:col0 + 1024],
                                            po[:, 512:1024],
                                            bias_sb[:, J:J + 1])
                if eng_dma is not None:
                    eng_dma.dma_start(dma_dst, dma_src)

            # chunk 0 has no cross term -> run as soon as x chunk 0 lands;
            # out0 goes on the otherwise-idle gpsimd ring
            out0 = osb.tile([C, R], F16, tag="os0")
            emit_main(0, out0, 0, nc.gpsimd, out0[:], outg_d[:, 0:R])

            # --- A-phase: 16 col-tiled matmuls per half into 4 PSUM strips
            a_ps = [psp.tile([128, 512], F32, tag=f"pa{h}", name=f"pa{h}")
                    for h in range(2)]

            def emit_a(I):
                g = I % 4
                for h in range(2):
                    nc.tensor.matmul(a_ps[h][32 * g:32 * (g + 1), :],
                                     uu_sb[:, 32 * I:32 * (I + 1)],
                                     xg[:, R * I + 512 * h:
                                        R * I + 512 * (h + 1)],
                                     start=(I <= 3), stop=(I >= NCH - 5),
                                     skip_group_check=True,
                                     tile_position=(0, 32 * g))

            for I in range(NCH - 2):      # chunks 0..14
                emit_a(I)

            # --- pass A: A_1..A_15 are final once chunks 0..14 are in.
            # Emitted BEFORE chunk 15's A matmuls so Tile orders the strip
            # reads first (WAR) and mains 1..15 unblock without waiting for
            # the last chunk.
            a4_sb = cst.tile([128, R], F16, tag="a4")
            a2q_sb = cst.tile([32, R], F16, tag="a2q")
            for h in range(2):
                if h == 0:
                    nc.scalar.activation(a4_sb[:, 0:512], a_ps[0][:], Ident)
                else:
                    nc.vector.tensor_copy(a4_sb[:, 512:1024], a_ps[1][:])
                ar = psp.tile([32, 512], F32, tag=f"pr{h}", name=f"ar{h}")
                nc.tensor.matmul(ar[:], predq_sb[:],
                                 a4_sb[:, 512 * h:512 * (h + 1)],
                                 start=True, stop=True)
                if h == 0:
                    nc.scalar.activation(a2q_sb[:, 0:512], ar[:], Ident)
                else:
                    nc.vector.tensor_copy(a2q_sb[:, 512:1024], ar[:])

            # scatter A_1..A_15 into xg partitions 0/1, split 4 ways over
            # the free rings (q-major a2q rows make iteration orders match)
            nc.sync.dma_start(xg[0:1, R:9 * R], a2q_sb[0:8, :])
            nc.sync.dma_start(xg[1:2, R:9 * R], a2q_sb[16:24, :])
            nc.gpsimd.dma_start(xg[0:1, 9 * R:16 * R], a2q_sb[8:15, :])
            nc.gpsimd.dma_start(xg[1:2, 9 * R:16 * R], a2q_sb[24:31, :])

            # --- chunk 15's A contribution + pass B for A_16.
            # A_16 lives on strip rows 0/1 (partitions 32g/32g+1 after the
            # J%16 remap).  Strips 0..2 got their last write at chunks
            # 12/13/14, so pass A's a4_sb copy already holds their final
            # values; chunk 15 only updates strip 3 — refresh partitions
            # 96/97 and reduce the four strip rows with a selector.
            emit_a(NCH - 2)
            a2b_sb = cst.tile([2, R], F16, tag="a2b")
            for h in range(2):
                if h == 0:
                    nc.scalar.activation(a4_sb[96:98, 0:512],
                                         a_ps[0][96:98, :], Ident)
                else:
                    nc.vector.tensor_copy(a4_sb[96:98, 512:1024],
                                          a_ps[1][96:98, :])
                arb = psp.tile([2, 512], F32, tag=f"pr{h}", name=f"arb{h}")
                nc.tensor.matmul(arb[:], predb_sb[:],
                                 a4_sb[:, 512 * h:512 * (h + 1)],
                                 start=True, stop=True)
                if h == 0:
                    nc.scalar.activation(a2b_sb[:, 0:512], arb[:], Ident)
                else:
                    nc.vector.tensor_copy(a2b_sb[:, 512:1024], arb[:])
            nc.gpsimd.dma_start(xg[0:2, 16 * R:17 * R], a2b_sb[:])

            # --- mains: per chunk one K=128 matmul per half + bias copy;
            # outputs flushed as 516KB pairs on sync/gpsimd rings
            for p in range(8):
                J0 = 1 + 2 * p
                pair = osb.tile([C, 2 * R], F16, tag="osb")
                emit_main(J0, pair, 0)
                eng = nc.sync if p % 2 == 0 else nc.gpsimd
                emit_main(J0 + 1, pair, R, eng,
                          pair[:], outg_d[:, R * J0:R * (J0 + 2)])

    nc.compile()
    return nc


def _host_prep(weight, bias, decay_value):
    w0 = np.zeros(C * NCH); w1 = np.zeros(C * NCH)
    w0[:S] = weight[0].astype(np.float64)
    w1[:S] = weight[1].astype(np.float64)
    d0 = float(np.clip(np.float32(decay_value[0, 0]), 0.9, 1.0))
    d1 = float(np.clip(np.float32(decay_value[1, 0]), 0.9, 1.0))
    sl = np.arange(C, dtype=np.float64)

    uu = np.zeros((128, 16 * 32), dtype=np.float16)
    with np.errstate(under='ignore'):
        for I in range(NCH - 1):
            for J in range(I + 1, NCH):
                e = (126.0 * (J - I) - sl) / DC
                m = 2 * (J % 16)        # J=16 -> strip rows 0/1
                uu[2:128, 32 * I + m] = (w0[C * I:C * (I + 1)] * d0 ** e
                                         ).astype(np.float16)
                uu[2:128, 32 * I + m + 1] = (d1 ** e).astype(np.float16)

        # host-side stationaries: wf[J][p, t] for chunk J
        #   p=0: d0^tl (decay row, J>0), p=1: w1-side decay row,
        #   p>=2: causal diag block w0[s] d0^(t-s) + w1[t] d1^(t-s)
        wfs = np.zeros((NCH, 128, C), dtype=np.float16)
        tl = sl
        causal = tl[None, :] >= sl[:, None]
        for J in range(NCH):
            c0 = C * J
            blk = (w0[c0:c0 + C, None] * d0 ** (tl[None, :] - sl[:, None])
                   + w1[None, c0:c0 + C] * d1 ** (tl[None, :] - sl[:, None]))
            wfs[J, 2:128, :] = np.where(causal, blk, 0.0).astype(np.float16)
            if J > 0:
                wfs[J, 0, :] = (d0 ** tl).astype(np.float16)
                wfs[J, 1, :] = (w1[c0:c0 + C] * d1 ** tl).astype(np.float16)

    # q-major strip reduce: a2q row 16q+(J-1) = sum_g a4[32g + 2(J%16) + q]
    predq = np.zeros((128, 32), dtype=np.float16)
    for g in range(4):
        for J in range(1, NCH):
            for q in range(2):
                predq[32 * g + 2 * (J % 16) + q, 16 * q + (J - 1)] = 1
    # A_16 selector: sum strip rows 0/1 across the four 32-partition strips
    predb = np.zeros((128, 2), dtype=np.float16)
    for g in range(4):
        for q in range(2):
            predb[32 * g + q, q] = 1

    p1 = np.zeros((128, 672), dtype=np.float16)
    p1[:, 0:512] = uu
    p1[:, 512:544] = predq
    p1[:, 544:546] = predb
    p1[:, 546:672] = wfs[0]
    wfa = np.ascontiguousarray(
        wfs[1:9].transpose(1, 0, 2).reshape(128, 8 * C))
    wfb = np.ascontiguousarray(
        wfs[9:17].transpose(1, 0, 2).reshape(128, 8 * C))

    biasT = np.zeros((C, NCH), dtype=np.float32)
    bias32 = bias.astype(np.float32)
    for J in range(NCH):
        hi = min(C, S - C * J)
        biasT[:hi, J] = bias32[C * J:C * J + hi]
    return p1, wfa, wfb, biasT


def make_in_maps(inputs):
    x = np.asarray(inputs["x"], dtype=np.float32)
    weight = np.asarray(inputs["weight"], dtype=np.float32)
    bias = np.asarray(inputs["bias"], dtype=np.float32)
    decay_value = np.asarray(inputs["decay_value"], dtype=np.float32)

    p1, wfa, wfb, biasT = _host_prep(weight, bias, decay_value)

    x16 = x.reshape(B * E, S).astype(np.float16)
    in_maps = []
    for c in range(N_CORES):
        xc = x16[R * c:R * (c + 1), :]                    # [R, S]
        xgc = np.zeros((128, NCH * R), dtype=np.float16)  # rows 0/1 zero
        xcT = xc.T                                        # [S, R]
        for J in range(NCH):
            hi = min(C, S - C * J)
            xgc[2:2 + hi, R * J:R * (J + 1)] = xcT[C * J:C * J + hi, :]
        in_maps.append({
            "xg": np.ascontiguousarray(xgc), "p1": p1,
            "wfa": wfa, "wfb": wfb, "biasT": biasT,
        })
    return in_maps


def kernel(x, weight, bias, decay_value, index=0, recurrent=0, **_):
    global _PROGRAM
    if _PROGRAM is None:
        _PROGRAM = _build_program()
    nc = _PROGRAM

    in_maps = make_in_maps({"x": x, "weight": weight, "bias": bias,
                            "decay_value": decay_value})

    res = run_bass_kernel_spmd(nc, in_maps, core_ids=list(range(N_CORES)))
    out = np.empty((B * E, S), dtype=np.float32)
    for c in range(N_CORES):
        og = np.asarray(res.results[c]["outg"])            # [C, NCH*R] f16
        ot = np.empty((S, R), dtype=np.float32)
        for J in range(NCH):
            hi = min(C, S - C * J)
            ot[C * J:C * J + hi, :] = og[0:hi, R * J:R * (J + 1)
                                         ].astype(np.float32)
        out[R * c:R * (c + 1), :] = ot.T
    return out.reshape(B, E, S)


# revision 12
# speedup vs baseline: 1.0994x; 1.0994x over previous
"""Trainium2 Bass kernel for CombinedRepeatCausalLinear (parallel forward).

Computes out[b,e,t] = sum_s x[b,e,s] * W[s,t] + bias[t] where
  W[s,t] = mask(t>=s) * (w0[s]*d0^(t-s) + w1[t]*d1^(t-s))
for S = 2048, x of shape (8, 1024, 2048) fp32.

Strategy (8 NeuronCores, data-parallel over batch; fp16 datapath):
  W is causal-masked rank-2.  Split s/t into 17 chunks of C=126.  For
  target chunk J the contribution of all s < 126J is exactly rank 2:
     out[t in J] = (diag block) + d0^tl * A0_J + w1[t] d1^tl * A1_J
  with A0_J[e] = sum_{s<126J} w0[s] d0^(126J-s) x[s,e]  (A1 analogous).
  C=126 leaves 2 spare K-rows, so the cross term folds into the SAME
  K=128 matmul as the 126x126 diagonal block: two moving-operand
  partitions carry the per-chunk A rows, the rest carry the x chunk.

  v4 layout: chunk J's A rows live on partition pair 2*((J-1)//4), so
  the post-reduce scatter is 4 contiguous 2-partition HWDGE DMAs (one
  per group of 4 consecutive chunks) instead of 1-partition crawls.
  Input arrives in 1MB 4-chunk groups on two HWDGE rings; stationaries
  are generated on device (K=2 matmul + mask multiply) pipelined with
  the per-chunk A-contribution matmuls under the load.  A_1..A_15 need
  only chunks 0..14, so their strip reduce + scatter is emitted BETWEEN
  chunk 14's and chunk 15's A matmuls; A_16 comes from a tiny pass B.
  Mains stream per chunk with PSUM->SBUF bias copies split ACT/DVE and
  paired 516KB output DMAs on the sync/gpsimd rings.
"""

import numpy as np

import concourse.bass as bass
import concourse.mybir as mybir
import concourse.tile as tile
from concourse import bacc
from concourse.bass_utils import run_bass_kernel_spmd

F16 = mybir.dt.float16
F32 = mybir.dt.float32

B = 8
E = 1024
S = 2048
DC = 1.0
N_CORES = 8
R = (B * E) // N_CORES      # rows (e) per core = 1024
C = 126                     # chunk size along s/t
NCH = 17                    # chunks; chunk 16 has only 32 valid rows
LAST = S - C * (NCH - 1)    # 32

_PROGRAM = None


def _mj(J):
    """A-pair group of chunk J (J>=1): partitions 2m/2m+1 hold its A rows.
    Chunk 0 has no A rows and uses the m=0 layout with zeros there."""
    return (J - 1) // 4 if J >= 1 else 0


def _xrows(J):
    """Partition index for local x row s (s=0..125) of chunk J."""
    m = _mj(J)
    p = np.arange(C) + 0
    return np.where(np.arange(C) < 2 * m, np.arange(C), np.arange(C) + 2)


def _a2row(J, q):
    """Row of a2q holding A_J component q (pass A: J=1..15)."""
    m = (J - 1) // 4
    cnt = 3 if m == 3 else 4
    return 8 * m + q * cnt + (J - 1) % 4


def _build_program():
    nc = bacc.Bacc("TRN2", target_bir_lowering=False, debug=False,
                   num_devices=N_CORES)

    xg_d = nc.declare_dram_parameter("xg", [128, NCH * R], F16, isOutput=False)
    # p1: mask(4*126) | uu(512) | predq(32) | predb(2) cols, fp16
    NP1 = 4 * C + 512 + 32 + 2
    p1_d = nc.declare_dram_parameter("p1", [128, NP1], F16, isOutput=False)
    # p2: fs(17*128) | fm(17*126) on 2 partitions
    NP2 = NCH * 128 + NCH * C
    p2_d = nc.declare_dram_parameter("p2", [2, NP2], F16, isOutput=False)
    biasT_d = nc.declare_dram_parameter("biasT", [C, NCH], F32,
                                        isOutput=False)
    outg_d = nc.declare_dram_parameter("outg", [C, NCH * R], F16,
                                       isOutput=True)

    Ident = mybir.ActivationFunctionType.Identity

    with tile.TileContext(nc) as tc:
        with (
            tc.tile_pool(name="cst", bufs=1) as cst,
            tc.tile_pool(name="xp", bufs=1) as xp,
            tc.tile_pool(name="wd", bufs=NCH) as wdp,
            tc.tile_pool(name="osb", bufs=3) as osb,
            tc.tile_pool(name="ps", bufs=1, space="PSUM") as psp,
            tc.tile_pool(name="po", bufs=3, space="PSUM") as pop,
        ):
            # --- params first on the HWDGE rings, then 1MB x groups
            p1_sb = cst.tile([128, NP1], F16, tag="p1")
            nc.sync.dma_start(p1_sb[:], p1_d[:])
            p2_sb = cst.tile([2, NP2], F16, tag="p2")
            nc.scalar.dma_start(p2_sb[:], p2_d[:])
            bias_sb = cst.tile([C, NCH], F32, tag="bias")
            nc.scalar.dma_start(bias_sb[:], biasT_d[:])
            mask_sb = p1_sb[:, 0:4 * C]
            uu_sb = p1_sb[:, 4 * C:4 * C + 512]
            predq_sb = p1_sb[:, 4 * C + 512:4 * C + 544]
            predb_sb = p1_sb[:, 4 * C + 544:4 * C + 546]
            fs_sb = p2_sb[:, 0:NCH * 128]
            fm_sb = p2_sb[:, NCH * 128:]

            xg = xp.tile([128, NCH * R], F16, tag="xg")
            groups = [(0, 4, nc.sync), (4, 8, nc.scalar), (8, 12, nc.sync),
                      (12, 16, nc.scalar), (16, 17, nc.sync)]
            for lo, hi, eng in groups:
                eng.dma_start(xg[:, lo * R:hi * R], xg_d[:, lo * R:hi * R])

            # --- on-device stationary generation: rank-2 + causal mask
            wf_sb = []

            def emit_wgen(J):
                pw = pop.tile([128, C], F32, tag="po", name=f"pw{J}")
                nc.tensor.matmul(pw[:], fs_sb[:, 128 * J:128 * (J + 1)],
                                 fm_sb[:, C * J:C * (J + 1)],
                                 start=True, stop=True)
                wf = wdp.tile([128, C], F16, tag="wd", name=f"wd{J}")
                nc.vector.tensor_mul(wf[:], pw[:],
                                     mask_sb[:, C * _mj(J):C * (_mj(J) + 1)])
                wf_sb.append(wf)

            def emit_main(J, out_sb, col0, eng_dma=None, dma_src=None,
                          dma_dst=None):
                po = pop.tile([C, R], F32, tag="po", name=f"po{J}")
                for h in range(2):
                    nc.tensor.matmul(po[:, 512 * h:512 * (h + 1)], wf_sb[J][:],
                                     xg[:, R * J + 512 * h:
                                        R * J + 512 * (h + 1)],
                                     start=True, stop=True)
                nc.scalar.activation(out_sb[:, col0:col0 + 512],
                                     po[:, 0:512], Ident,
                                     bias=bias_sb[:, J:J + 1])
                nc.vector.tensor_scalar_add(out_sb[:, col0 + 512:col0 + 1024],
                                            po[:, 512:1024],
                                            bias_sb[:, J:J + 1])
                if eng_dma is not None:
                    eng_dma.dma_start(dma_dst, dma_src)

            # --- A-phase strips
            a_ps = [psp.tile([128, 512], F32, tag=f"pa{h}", name=f"pa{h}")
                    for h in range(2)]

            def emit_a(I):
                g = I % 4
                for h in range(2):
                    nc.tensor.matmul(a_ps[h][32 * g:32 * (g + 1), :],
                                     uu_sb[:, 32 * I:32 * (I + 1)],
                                     xg[:, R * I + 512 * h:
                                        R * I + 512 * (h + 1)],
                                     start=(I <= 3), stop=(I >= NCH - 5),
                                     skip_group_check=True,
                                     tile_position=(0, 32 * g))

            # interleave wgen with per-group A matmuls; main0 + out0 early
            emit_wgen(0)
            out0 = osb.tile([C, R], F16, tag="os0")
            emit_main(0, out0, 0, nc.gpsimd, out0[:], outg_d[:, 0:R])
            for J in range(1, 5):
                emit_wgen(J)
            for I in range(0, 4):
                emit_a(I)
            for J in range(5, 9):
                emit_wgen(J)
            for I in range(4, 8):
                emit_a(I)
            for J in range(9, 13):
                emit_wgen(J)
            for I in range(8, 12):
                emit_a(I)
            for J in range(13, NCH):
                emit_wgen(J)
            for I in range(12, NCH - 2):
                emit_a(I)

            # --- pass A: A_1..A_15 are final once chunks 0..14 are in.
            # Emitted BEFORE chunk 15's A matmuls so Tile orders the strip
            # reads first (WAR) and mains 1..15 unblock without waiting for
            # the last chunk.
            a4_sb = cst.tile([128, R], F16, tag="a4")
            a2q_sb = cst.tile([32, R], F16, tag="a2q")
            for h in range(2):
                if h == 0:
                    nc.scalar.activation(a4_sb[:, 0:512], a_ps[0][:], Ident)
                else:
                    nc.vector.tensor_copy(a4_sb[:, 512:1024], a_ps[1][:])
                ar = pop.tile([32, 512], F32, tag="po", name=f"ar{h}")
                nc.tensor.matmul(ar[:], predq_sb[:],
                                 a4_sb[:, 512 * h:512 * (h + 1)],
                                 start=True, stop=True)
                if h == 0:
                    nc.scalar.activation(a2q_sb[:, 0:512], ar[:], Ident)
                else:
                    nc.vector.tensor_copy(a2q_sb[:, 512:1024], ar[:])

            # scatter A_1..A_15: one contiguous 2-partition DMA per group
            # of 4 consecutive chunks (a2q rows are grouped to match)
            nc.sync.dma_start(xg[0:2, R:5 * R], a2q_sb[0:8, :])
            nc.scalar.dma_start(xg[2:4, 5 * R:9 * R], a2q_sb[8:16, :])
            nc.sync.dma_start(xg[4:6, 9 * R:13 * R], a2q_sb[16:24, :])
            nc.scalar.dma_start(xg[6:8, 13 * R:16 * R], a2q_sb[24:30, :])

            # --- chunk 15's A contribution + pass B for A_16.
            # A_16 lives on strip rows 0/1 (J%16 remap).  Strips 0..2 are
            # final at pass A; chunk 15 only updates strip 3 — refresh
            # partitions 96/97 and reduce with a selector stationary.
            emit_a(NCH - 2)
            a2b_sb = cst.tile([2, R], F16, tag="a2b")
            for h in range(2):
                if h == 0:
                    nc.scalar.activation(a4_sb[96:98, 0:512],
                                         a_ps[0][96:98, :], Ident)
                else:
                    nc.vector.tensor_copy(a4_sb[96:98, 512:1024],
                                          a_ps[1][96:98, :])
                arb = pop.tile([2, 512], F32, tag="po", name=f"arb{h}")
                nc.tensor.matmul(arb[:], predb_sb[:],
                                 a4_sb[:, 512 * h:512 * (h + 1)],
                                 start=True, stop=True)
                if h == 0:
                    nc.scalar.activation(a2b_sb[:, 0:512], arb[:], Ident)
                else:
                    nc.vector.tensor_copy(a2b_sb[:, 512:1024], arb[:])
            nc.gpsimd.dma_start(xg[6:8, 16 * R:17 * R], a2b_sb[:])

            # --- mains: per chunk one K=128 matmul per half + bias copy;
            # outputs flushed as 516KB pairs on sync/gpsimd rings
            for p in range(8):
                J0 = 1 + 2 * p
                pair = osb.tile([C, 2 * R], F16, tag="osb")
                emit_main(J0, pair, 0)
                eng = nc.sync if p % 2 == 0 else nc.gpsimd
                emit_main(J0 + 1, pair, R, eng,
                          pair[:], outg_d[:, R * J0:R * (J0 + 2)])

    nc.compile()
    return nc


def _host_prep(weight, bias, decay_value):
    w0 = np.zeros(C * NCH); w1 = np.zeros(C * NCH)
    w0[:S] = weight[0].astype(np.float64)
    w1[:S] = weight[1].astype(np.float64)
    d0 = float(np.clip(np.float32(decay_value[0, 0]), 0.9, 1.0))
    d1 = float(np.clip(np.float32(decay_value[1, 0]), 0.9, 1.0))
    sl = np.arange(C, dtype=np.float64)

    uu = np.zeros((128, 16 * 32), dtype=np.float16)
    fs = np.zeros((2, NCH * 128), dtype=np.float16)
    fm = np.zeros((2, NCH * C), dtype=np.float16)
    with np.errstate(under='ignore'):
        for I in range(NCH - 1):
            rows = _xrows(I)            # partition of local x row s
            for J in range(I + 1, NCH):
                e = (126.0 * (J - I) - sl) / DC
                m2 = 2 * (J % 16)       # J=16 -> strip rows 0/1
                uu[rows, 32 * I + m2] = (w0[C * I:C * (I + 1)] * d0 ** e
                                         ).astype(np.float16)
                uu[rows, 32 * I + m2 + 1] = (d1 ** e).astype(np.float16)
        for J in range(NCH):
            c0 = C * J
            rows = _xrows(J)
            # stationary factor rows: decay rows at the chunk's A pair,
            # x rows carry the diag-block factors
            if J > 0:
                ap = 2 * _mj(J)
                fs[0, 128 * J + ap] = np.float16(d0 ** (63.0 / DC))
                fs[1, 128 * J + ap + 1] = np.float16(d1 ** (63.0 / DC))
            fs[0, 128 * J + rows] = (
                w0[c0:c0 + C] * d0 ** ((63.0 - sl) / DC)).astype(np.float16)
            fs[1, 128 * J + rows] = (
                d1 ** ((63.0 - sl) / DC)).astype(np.float16)
            fm[0, c0:c0 + C] = (d0 ** ((sl - 63.0) / DC)).astype(np.float16)
            fm[1, c0:c0 + C] = (w1[c0:c0 + C] * d1 ** ((sl - 63.0) / DC)
                                ).astype(np.float16)
        fm[:, C * 16 + LAST:] = 0

    # per-m causal masks: A pair rows all-ones, x rows causal
    mask = np.zeros((128, 4 * C), dtype=np.float16)
    causal = (sl[None, :] >= sl[:, None]).astype(np.float16)
    for m in range(4):
        mask[2 * m:2 * m + 2, C * m:C * (m + 1)] = 1
        rows = np.where(np.arange(C) < 2 * m, np.arange(C),
                        np.arange(C) + 2)
        mask[rows, C * m:C * (m + 1)] = causal

    # strip reduce: a2q row _a2row(J,q) = sum_g a4[32g + 2(J%16) + q]
    predq = np.zeros((128, 32), dtype=np.float16)
    for g in range(4):
        for J in range(1, NCH - 1):
            for q in range(2):
                predq[32 * g + 2 * (J % 16) + q, _a2row(J, q)] = 1
    # A_16 selector: sum strip rows 0/1 across the four strips
    predb = np.zeros((128, 2), dtype=np.float16)
    for g in range(4):
        for q in range(2):
            predb[32 * g + q, q] = 1

    p1 = np.zeros((128, 4 * C + 546), dtype=np.float16)
    p1[:, 0:4 * C] = mask
    p1[:, 4 * C:4 * C + 512] = uu
    p1[:, 4 * C + 512:4 * C + 544] = predq
    p1[:, 4 * C + 544:4 * C + 546] = predb
    p2 = np.concatenate([fs, fm], axis=1)

    biasT = np.zeros((C, NCH), dtype=np.float32)
    bias32 = bias.astype(np.float32)
    for J in range(NCH):
        hi = min(C, S - C * J)
        biasT[:hi, J] = bias32[C * J:C * J + hi]
    return p1, p2, biasT


def make_in_maps(inputs):
    x = np.asarray(inputs["x"], dtype=np.float32)
    weight = np.asarray(inputs["weight"], dtype=np.float32)
    bias = np.asarray(inputs["bias"], dtype=np.float32)
    decay_value = np.asarray(inputs["decay_value"], dtype=np.float32)

    p1, p2, biasT = _host_prep(weight, bias, decay_value)

    x16 = x.reshape(B * E, S).astype(np.float16)
    in_maps = []
    for c in range(N_CORES):
        xc = x16[R * c:R * (c + 1), :]                    # [R, S]
        xgc = np.zeros((128, NCH * R), dtype=np.float16)
        xcT = xc.T                                        # [S, R]
        for J in range(NCH):
            hi = min(C, S - C * J)
            rows = _xrows(J)[:hi]
            xgc[np.ix_(rows, np.arange(R * J, R * (J + 1)))] = \
                xcT[C * J:C * J + hi, :]
        in_maps.append({
            "xg": np.ascontiguousarray(xgc), "p1": p1, "p2": p2,
            "biasT": biasT,
        })
    return in_maps


def kernel(x, weight, bias, decay_value, index=0, recurrent=0, **_):
    global _PROGRAM
    if _PROGRAM is None:
        _PROGRAM = _build_program()
    nc = _PROGRAM

    in_maps = make_in_maps({"x": x, "weight": weight, "bias": bias,
                            "decay_value": decay_value})

    res = run_bass_kernel_spmd(nc, in_maps, core_ids=list(range(N_CORES)))
    out = np.empty((B * E, S), dtype=np.float32)
    for c in range(N_CORES):
        og = np.asarray(res.results[c]["outg"])            # [C, NCH*R] f16
        ot = np.empty((S, R), dtype=np.float32)
        for J in range(NCH):
            hi = min(C, S - C * J)
            ot[C * J:C * J + hi, :] = og[0:hi, R * J:R * (J + 1)
                                         ].astype(np.float32)
        out[R * c:R * (c + 1), :] = ot.T
    return out.reshape(B, E, S)


# revision 13
# speedup vs baseline: 1.2351x; 1.1234x over previous
"""Trainium2 Bass kernel for CombinedRepeatCausalLinear (parallel forward).

Computes out[b,e,t] = sum_s x[b,e,s] * W[s,t] + bias[t] where
  W[s,t] = mask(t>=s) * (w0[s]*d0^(t-s) + w1[t]*d1^(t-s))
for S = 2048, x of shape (8, 1024, 2048) fp32.

Strategy (8 NeuronCores, data-parallel over batch; fp16 datapath):
  W is causal-masked rank-2.  Split s/t into 17 chunks of C=126.  For
  target chunk J the contribution of all s < 126J is exactly rank 2:
     out[t in J] = (diag block) + d0^tl * A0_J + w1[t] d1^tl * A1_J
  with A0_J[e] = sum_{s<126J} w0[s] d0^(126J-s) x[s,e]  (A1 analogous).
  C=126 leaves 2 spare K-rows, so the cross term folds into the SAME
  K=128 matmul as the 126x126 diagonal block: two moving-operand
  partitions carry the per-chunk A rows, the rest carry the x chunk.

  v5: all stationaries host-computed (on-device wgen serializes the
  in-order Tensor queue).  Chunk J's A rows live on partition pair
  2*((J-1)//4) so the scatter is 4 contiguous 2-partition HWDGE DMAs.
  Input arrives in 1MB 4-chunk groups on two HWDGE rings with the
  per-chunk A-contribution matmuls (4-strip col-tiled PSUM
  accumulation) hidden under the load.  The strip reduce runs in two
  passes: A_1..A_12 right after chunk 11 (so mains 1..12 overlap the
  input tail), A_13..A_15 after chunk 14, A_16 via a selector matmul
  after chunk 15 — ordered so no matmul ever blocks the in-order
  Tensor queue on a far dependency.  Main bias copies use SEPARATE
  per-half PSUM tensors so Tile's bank tracker doesn't serialize the
  ACT/DVE halves.  Output flushes as 516KB pairs on sync/gpsimd rings.
"""

import numpy as np

import concourse.bass as bass
import concourse.mybir as mybir
import concourse.tile as tile
from concourse import bacc
from concourse.bass_utils import run_bass_kernel_spmd

F16 = mybir.dt.float16
F32 = mybir.dt.float32

B = 8
E = 1024
S = 2048
DC = 1.0
N_CORES = 8
R = (B * E) // N_CORES      # rows (e) per core = 1024
C = 126                     # chunk size along s/t
NCH = 17                    # chunks; chunk 16 has only 32 valid rows
LAST = S - C * (NCH - 1)    # 32

_PROGRAM = None


def _mj(J):
    """A-pair group of chunk J (J>=1): partitions 2m/2m+1 hold its A rows.
    Chunk 0 has no A rows and uses the m=0 layout with zeros there."""
    return (J - 1) // 4 if J >= 1 else 0


def _xrows(J):
    """Partition index for local x row s (s=0..125) of chunk J."""
    m = _mj(J)
    return np.where(np.arange(C) < 2 * m, np.arange(C), np.arange(C) + 2)


def _a2row(J, q):
    """Row of a2q holding A_J component q (J=1..15)."""
    m = (J - 1) // 4
    cnt = 3 if m == 3 else 4
    return 8 * m + q * cnt + (J - 1) % 4


def _build_program():
    nc = bacc.Bacc("TRN2", target_bir_lowering=False, debug=False,
                   num_devices=N_CORES)

    xg_d = nc.declare_dram_parameter("xg", [128, NCH * R], F16, isOutput=False)
    # p1: uu(512) | predq(32) | predb(2) | wf0(126) cols, fp16
    NP1 = 512 + 32 + 2 + C
    p1_d = nc.declare_dram_parameter("p1", [128, NP1], F16, isOutput=False)
    wfa_d = nc.declare_dram_parameter("wfa", [128, 8 * C], F16, isOutput=False)
    wfb_d = nc.declare_dram_parameter("wfb", [128, 8 * C], F16, isOutput=False)
    biasT_d = nc.declare_dram_parameter("biasT", [C, NCH], F32,
                                        isOutput=False)
    outg_d = nc.declare_dram_parameter("outg", [C, NCH * R], F16,
                                       isOutput=True)

    Ident = mybir.ActivationFunctionType.Identity

    with tile.TileContext(nc) as tc:
        with (
            tc.tile_pool(name="cst", bufs=1) as cst,
            tc.tile_pool(name="xp", bufs=1) as xp,
            tc.tile_pool(name="osb", bufs=3) as osb,
            tc.tile_pool(name="ps", bufs=1, space="PSUM") as psp,
            tc.tile_pool(name="p0", bufs=3, space="PSUM") as pop0,
            tc.tile_pool(name="p1", bufs=3, space="PSUM") as pop1,
        ):
            # --- params first on the HWDGE rings, then 1MB x groups
            p1_sb = cst.tile([128, NP1], F16, tag="p1")
            nc.sync.dma_start(p1_sb[:], p1_d[:])
            bias_sb = cst.tile([C, NCH], F32, tag="bias")
            nc.scalar.dma_start(bias_sb[:], biasT_d[:])
            uu_sb = p1_sb[:, 0:512]
            predq_sb = p1_sb[:, 512:544]
            predb_sb = p1_sb[:, 544:546]
            wf0_sb = p1_sb[:, 546:546 + C]

            xg = xp.tile([128, NCH * R], F16, tag="xg")
            groups = [(0, 4, nc.sync), (4, 8, nc.scalar), (8, 12, nc.sync),
                      (12, 16, nc.scalar), (16, 17, nc.sync)]
            for lo, hi, eng in groups:
                eng.dma_start(xg[:, lo * R:hi * R], xg_d[:, lo * R:hi * R])
            wfa_sb = cst.tile([128, 8 * C], F16, tag="wfa")
            nc.scalar.dma_start(wfa_sb[:], wfa_d[:])
            wfb_sb = cst.tile([128, 8 * C], F16, tag="wfb")
            nc.scalar.dma_start(wfb_sb[:], wfb_d[:])

            def wf(J):
                if J == 0:
                    return wf0_sb
                if J <= 8:
                    return wfa_sb[:, (J - 1) * C:J * C]
                return wfb_sb[:, (J - 9) * C:(J - 8) * C]

            def emit_main(J, out_sb, col0, eng_dma=None, dma_src=None,
                          dma_dst=None):
                # separate per-half PSUM tensors: the ACT and DVE copies
                # are independent for Tile's bank tracker and overlap
                poh = [pop0.tile([C, 512], F32, tag="po0", name=f"po{J}h0"),
                       pop1.tile([C, 512], F32, tag="po1", name=f"po{J}h1")]
                for h in range(2):
                    nc.tensor.matmul(poh[h][:], wf(J),
                                     xg[:, R * J + 512 * h:
                                        R * J + 512 * (h + 1)],
                                     start=True, stop=True)
                nc.scalar.activation(out_sb[:, col0:col0 + 512],
                                     poh[0][:], Ident,
                                     bias=bias_sb[:, J:J + 1])
                nc.vector.tensor_scalar_add(out_sb[:, col0 + 512:col0 + 1024],
                                            poh[1][:],
                                            bias_sb[:, J:J + 1])
                if eng_dma is not None:
                    eng_dma.dma_start(dma_dst, dma_src)

            # --- A-phase strips
            a_ps = [psp.tile([128, 512], F32, tag=f"pa{h}", name=f"pa{h}")
                    for h in range(2)]

            def emit_a(I):
                g = I % 4
                for h in range(2):
                    nc.tensor.matmul(a_ps[h][32 * g:32 * (g + 1), :],
                                     uu_sb[:, 32 * I:32 * (I + 1)],
                                     xg[:, R * I + 512 * h:
                                        R * I + 512 * (h + 1)],
                                     start=(I <= 3), stop=(I >= NCH - 5),
                                     skip_group_check=True,
                                     tile_position=(0, 32 * g))

            def emit_reduce(a4t, a2t, name):
                """Copy strips to SBUF and reduce to per-J A rows."""
                for h in range(2):
                    if h == 0:
                        nc.scalar.activation(a4t[:, 0:512], a_ps[0][:], Ident)
                    else:
                        nc.vector.tensor_copy(a4t[:, 512:1024], a_ps[1][:])
                    ar = pop0.tile([32, 512], F32, tag="po0",
                                   name=f"{name}h{h}")
                    nc.tensor.matmul(ar[:], predq_sb[:],
                                     a4t[:, 512 * h:512 * (h + 1)],
                                     start=True, stop=True)
                    if h == 0:
                        nc.scalar.activation(a2t[:, 0:512], ar[:], Ident)
                    else:
                        nc.vector.tensor_copy(a2t[:, 512:1024], ar[:])

            # chunk 0 main + out0 on the idle gpsimd ring, early
            out0 = osb.tile([C, R], F16, tag="os0")
            emit_main(0, out0, 0, nc.gpsimd, out0[:], outg_d[:, 0:R])

            for I in range(0, 12):        # groups G0..G2
                emit_a(I)

            # --- pass A1: A_1..A_12 final once chunks 0..11 are in
            a4a_sb = cst.tile([128, R], F16, tag="a4a")
            a2qa_sb = cst.tile([32, R], F16, tag="a2qa")
            emit_reduce(a4a_sb, a2qa_sb, "ar1")
            nc.sync.dma_start(xg[0:2, R:5 * R], a2qa_sb[0:8, :])
            nc.scalar.dma_start(xg[2:4, 5 * R:9 * R], a2qa_sb[8:16, :])
            nc.sync.dma_start(xg[4:6, 9 * R:13 * R], a2qa_sb[16:24, :])

            # --- mains 1..12 (overlap the input tail); paired 516KB outs
            for p in range(6):
                J0 = 1 + 2 * p
                pair = osb.tile([C, 2 * R], F16, tag="osb")
                emit_main(J0, pair, 0)
                eng = nc.sync if p % 2 == 0 else nc.gpsimd
                emit_main(J0 + 1, pair, R, eng,
                          pair[:], outg_d[:, R * J0:R * (J0 + 2)])

            # --- chunks 12..14 strip contributions + pass A2 (A_13..A_15)
            for I in range(12, 15):
                emit_a(I)
            a4b_sb = cst.tile([128, R], F16, tag="a4b")
            a2qb_sb = cst.tile([32, R], F16, tag="a2qb")
            emit_reduce(a4b_sb, a2qb_sb, "ar2")
            nc.scalar.dma_start(xg[6:8, 13 * R:16 * R], a2qb_sb[24:30, :])

            # --- mains 13..15
            pair7 = osb.tile([C, 2 * R], F16, tag="osb")
            emit_main(13, pair7, 0)
            emit_main(14, pair7, R, nc.sync,
                      pair7[:], outg_d[:, 13 * R:15 * R])
            pair8 = osb.tile([C, 2 * R], F16, tag="osb")
            emit_main(15, pair8, 0)

            # --- chunk 15's strip write + pass B for A_16: strips 0..2
            # are final in a4b; refresh strip 3 rows (96/97) post chunk 15
            # and reduce with the selector stationary.
            emit_a(15)
            a2b_sb = cst.tile([2, R], F16, tag="a2b")
            for h in range(2):
                if h == 0:
                    nc.scalar.activation(a4b_sb[96:98, 0:512],
                                         a_ps[0][96:98, :], Ident)
                else:
                    nc.vector.tensor_copy(a4b_sb[96:98, 512:1024],
                                          a_ps[1][96:98, :])
                arb = pop1.tile([2, 512], F32, tag="po1", name=f"arb{h}")
                nc.tensor.matmul(arb[:], predb_sb[:],
                                 a4b_sb[:, 512 * h:512 * (h + 1)],
                                 start=True, stop=True)
                if h == 0:
                    nc.scalar.activation(a2b_sb[:, 0:512], arb[:], Ident)
                else:
                    nc.vector.tensor_copy(a2b_sb[:, 512:1024], arb[:])
            nc.gpsimd.dma_start(xg[6:8, 16 * R:17 * R], a2b_sb[:])

            # --- main 16 last (smallest output tail)
            emit_main(16, pair8, R, nc.gpsimd,
                      pair8[:], outg_d[:, 15 * R:17 * R])

    nc.compile()
    return nc


def _host_prep(weight, bias, decay_value):
    w0 = np.zeros(C * NCH); w1 = np.zeros(C * NCH)
    w0[:S] = weight[0].astype(np.float64)
    w1[:S] = weight[1].astype(np.float64)
    d0 = float(np.clip(np.float32(decay_value[0, 0]), 0.9, 1.0))
    d1 = float(np.clip(np.float32(decay_value[1, 0]), 0.9, 1.0))
    sl = np.arange(C, dtype=np.float64)

    uu = np.zeros((128, 16 * 32), dtype=np.float16)
    wfs = np.zeros((NCH, 128, C), dtype=np.float16)
    with np.errstate(under='ignore'):
        for I in range(NCH - 1):
            rows = _xrows(I)
            for J in range(I + 1, NCH):
                e = (126.0 * (J - I) - sl) / DC
                m2 = 2 * (J % 16)       # J=16 -> strip rows 0/1
                uu[rows, 32 * I + m2] = (w0[C * I:C * (I + 1)] * d0 ** e
                                         ).astype(np.float16)
                uu[rows, 32 * I + m2 + 1] = (d1 ** e).astype(np.float16)

        tl = sl
        causal = tl[None, :] >= sl[:, None]
        for J in range(NCH):
            c0 = C * J
            rows = _xrows(J)
            blk = (w0[c0:c0 + C, None] * d0 ** (tl[None, :] - sl[:, None])
                   + w1[None, c0:c0 + C] * d1 ** (tl[None, :] - sl[:, None]))
            blk = np.where(causal, blk, 0.0)
            wfs[J][rows, :] = blk.astype(np.float16)
            if J > 0:
                ap = 2 * _mj(J)
                wfs[J, ap, :] = (d0 ** tl).astype(np.float16)
                wfs[J, ap + 1, :] = (w1[c0:c0 + C] * d1 ** tl
                                     ).astype(np.float16)

    # strip reduce: a2q row _a2row(J,q) = sum_g a4[32g + 2(J%16) + q]
    predq = np.zeros((128, 32), dtype=np.float16)
    for g in range(4):
        for J in range(1, NCH - 1):
            for q in range(2):
                predq[32 * g + 2 * (J % 16) + q, _a2row(J, q)] = 1
    # A_16 selector: sum strip rows 0/1 across the four strips
    predb = np.zeros((128, 2), dtype=np.float16)
    for g in range(4):
        for q in range(2):
            predb[32 * g + q, q] = 1

    p1 = np.zeros((128, 512 + 32 + 2 + C), dtype=np.float16)
    p1[:, 0:512] = uu
    p1[:, 512:544] = predq
    p1[:, 544:546] = predb
    p1[:, 546:546 + C] = wfs[0]
    wfa = np.ascontiguousarray(
        wfs[1:9].transpose(1, 0, 2).reshape(128, 8 * C))
    wfb = np.ascontiguousarray(
        wfs[9:17].transpose(1, 0, 2).reshape(128, 8 * C))

    biasT = np.zeros((C, NCH), dtype=np.float32)
    bias32 = bias.astype(np.float32)
    for J in range(NCH):
        hi = min(C, S - C * J)
        biasT[:hi, J] = bias32[C * J:C * J + hi]
    return p1, wfa, wfb, biasT


def make_in_maps(inputs):
    x = np.asarray(inputs["x"], dtype=np.float32)
    weight = np.asarray(inputs["weight"], dtype=np.float32)
    bias = np.asarray(inputs["bias"], dtype=np.float32)
    decay_value = np.asarray(inputs["decay_value"], dtype=np.float32)

    p1, wfa, wfb, biasT = _host_prep(weight, bias, decay_value)

    x16 = x.reshape(B * E, S).astype(np.float16)
    in_maps = []
    for c in range(N_CORES):
        xc = x16[R * c:R * (c + 1), :]                    # [R, S]
        xgc = np.zeros((128, NCH * R), dtype=np.float16)
        xcT = xc.T                                        # [S, R]
        for J in range(NCH):
            hi = min(C, S - C * J)
            rows = _xrows(J)[:hi]
            xgc[np.ix_(rows, np.arange(R * J, R * (J + 1)))] = \
                xcT[C * J:C * J + hi, :]
        in_maps.append({
            "xg": np.ascontiguousarray(xgc), "p1": p1, "wfa": wfa,
            "wfb": wfb, "biasT": biasT,
        })
    return in_maps


def kernel(x, weight, bias, decay_value, index=0, recurrent=0, **_):
    global _PROGRAM
    if _PROGRAM is None:
        _PROGRAM = _build_program()
    nc = _PROGRAM

    in_maps = make_in_maps({"x": x, "weight": weight, "bias": bias,
                            "decay_value": decay_value})

    res = run_bass_kernel_spmd(nc, in_maps, core_ids=list(range(N_CORES)))
    out = np.empty((B * E, S), dtype=np.float32)
    for c in range(N_CORES):
        og = np.asarray(res.results[c]["outg"])            # [C, NCH*R] f16
        ot = np.empty((S, R), dtype=np.float32)
        for J in range(NCH):
            hi = min(C, S - C * J)
            ot[C * J:C * J + hi, :] = og[0:hi, R * J:R * (J + 1)
                                         ].astype(np.float32)
        out[R * c:R * (c + 1), :] = ot.T
    return out.reshape(B, E, S)
